# revision 17
# baseline (speedup 1.0000x reference)
"""Multi-head attention (B=4, L=2048, D=1024, H=16) on 8 trn2 NeuronCores.

Sharding: core = b*2 + hg  ->  batch sample b in 0..3, head-group hg in 0..1.
Each core handles one batch sample and 8 heads (512 of the 1024 model dims):
data parallel on B, tensor parallel on H (column-split w_q/w_k/w_v, row-split
w_o).  Each core produces a partial output (its 8 heads' contribution through
w_o); the host sums the two head-group partials per sample.

Device-side layouts are all transposed (contraction dim on partitions) so the
kernel needs no on-device transposes:
  xT   [D=1024, L=2048]  (host pre-transposes q/k/v per sample)
  QT/KT [512, 2048]      head-pair tiles: partitions 0:64 head 2p, 64:128 head 2p+1
  V    natural [L, 512]  stored per l-tile as [128, 8 heads, 65] with a ones
                         column appended per head (row sums of P for free)
  S^T  [keys, queries]   per (pair, m-block) in PSUM -> exp on ACT -> P^T bf16
  O^T  = V^T @ P^T accumulated in PSUM; row 64 = sum_n P^T[n, m] (softmax denom)
  outT [1024, 2048]      final partial, host transposes back
Softmax skips max-subtraction: scaled scores are ~N(0,1) (the 1/8 scale is
applied inside the ACT exp); |score| < ~7 over 33M samples, far from fp32 exp
overflow.

The attention loop is ONE GLOBAL software pipeline across all 16 (mb, pair)
units AND across KREP rep boundaries: a pending-window of pv-groups survives
pair/mb boundaries (each pair's PVs drain during the NEXT pair's score
groups, lag ~8 groups), the last pair's pv-groups and the final
out-projection carry into the next rep's K-projection phase, so the PE never
sits in a pipeline flush and ACT (the bottleneck engine: 256 exps of
[128,1024] ~= 322us measured via the nopv variant; the kernel runs at
~256 x 1117ns = the ACT floor at light load) is never starved at
boundaries. V/Q projections and out-projections are injected as filler units
between score groups (legal at every group boundary: with 2-chunk supertiles
the "st" PSUM tag is always flushed there). Score chunks ([128,512] matmul
outputs) accumulate into 3 rotating [128,1024] PSUM tiles (6 banks) shared
with the projection/out-projection accumulators; the PV accumulators take
the remaining 2 banks. The K projection is interleaved with the first pair's
score groups (group g only needs K-block g//2) so ACT starts ~50us earlier.
DMAs are batched 3-D loads ([128, ndt, .] views; one dma_start each for w/x
blocks) and staging/output DMAs issue from the idle Pool sequencer: SP-SEQ
dma_start costs ~565ns+625ns HWDGE each, and the old per-tile loads put
~240us of serialized issue time on SP (172 -> 45 dma instructions).
fp8 (DoubleRow or plain) was evaluated and is numerically DEAD here:
attention outputs are weighted means, so per-element quantization noise
(~6-9% for e4m3) passes ~1:1 to relative output error (measured 4-8e-2
vs the 2e-2 gate). Everything stays bf16 with fp32 PSUM accumulation.
"""

import os
import numpy as np
import ml_dtypes
from contextlib import ExitStack

import concourse.bass as bass
import concourse.tile as tile
from concourse import bacc, mybir
from concourse.bass import ts
from concourse.bass_utils import run_bass_kernel_spmd

B, L, D, H = 4, 2048, 1024, 16
DK = 64          # head dim
HLOC = 8         # heads per core
DL = 512         # model dims per core (HLOC * DK)
SCALE = 1.0 / 8.0
NCORES = 8

COMPUTE_DT = os.environ.get("KDT", "bf16")   # 'bf16' | 'f32'
VARIANT = os.environ.get("KVARIANT", "full")  # 'full' | 'proj' | 'nopv'
KREP = int(os.environ.get("KREP", "1"))       # body repetitions (timing only)

_MDT = {"bf16": mybir.dt.bfloat16, "f32": mybir.dt.float32}
_NPDT = {"bf16": ml_dtypes.bfloat16, "f32": np.float32}


def _build(nc, l=L, krep=None, variant=None):
    global KREP, VARIANT
    if krep is not None:
        KREP = krep
    if variant is not None:
        VARIANT = variant
    cdt = _MDT[COMPUTE_DT]
    f32 = mybir.dt.float32
    nlt = l // 128       # l-tiles of 128
    nlb = l // 512       # l-blocks of 512
    ndt = D // 128       # contraction d-tiles for projections

    xqT = nc.dram_tensor("xqT", [D, l], cdt, kind="ExternalInput").ap()
    xkT = nc.dram_tensor("xkT", [D, l], cdt, kind="ExternalInput").ap()
    xvT = nc.dram_tensor("xvT", [D, l], cdt, kind="ExternalInput").ap()
    wqT = nc.dram_tensor("wqT", [D, DL], cdt, kind="ExternalInput").ap()
    wkT = nc.dram_tensor("wkT", [D, DL], cdt, kind="ExternalInput").ap()
    wvT = nc.dram_tensor("wvT", [D, DL], cdt, kind="ExternalInput").ap()
    woT = nc.dram_tensor("woT", [DL, D], cdt, kind="ExternalInput").ap()
    outT = nc.dram_tensor("outT", [D, l], cdt, kind="ExternalOutput").ap()

    with tile.TileContext(nc) as tc, ExitStack() as ctx:
        persist = ctx.enter_context(tc.tile_pool(name="persist", bufs=1))
        wpool = ctx.enter_context(tc.tile_pool(name="wpool", bufs=2))
        xpool = ctx.enter_context(tc.tile_pool(name="xpool", bufs=2))
        ppool = ctx.enter_context(tc.tile_pool(name="ppool", bufs=4))
        otpool = ctx.enter_context(tc.tile_pool(name="otpool", bufs=7))
        opool = ctx.enter_context(tc.tile_pool(name="opool", bufs=3))
        small = ctx.enter_context(tc.tile_pool(name="small", bufs=4))
        # 2 rotating [128,1536] f32 supertiles (6 banks): score chunks,
        # projection and out-projection accumulators all share this tag.
        ps_s = ctx.enter_context(tc.tile_pool(name="ps_s", bufs=2, space="PSUM"))
        ps_pv = ctx.enter_context(tc.tile_pool(name="ps_pv", bufs=2, space="PSUM"))

        # default: 3 x [128,1024] score tiles, exp per 2 chunks. The wider
        # 2 x [128,1536] / exp-per-3 layout ("w" variant suffix) cuts ACT
        # instruction overhead ~25% but measured slower end-to-end (the
        # 2-buffer PSUM rotation couples PE to exp latency too tightly).
        wide = VARIANT.endswith("w")
        s_shape = [128, 1536] if wide else [128, 1024]
        s_bufs = 2 if wide else 3
        s_cap = 3 if wide else 2

        def spsum(name):
            return ps_s.tile(s_shape, f32, tag="st", bufs=s_bufs, name=name)

        # persistent tiles
        QT = [persist.tile([128, l], cdt, tag=f"qt{i}", name=f"qt{i}") for i in range(4)]
        KT = [persist.tile([128, l], cdt, tag=f"kt{i}", name=f"kt{i}") for i in range(4)]
        V = [persist.tile([128, HLOC, DK + 1], cdt, tag=f"v{i}", name=f"v{i}")
             for i in range(nlt)]
        # w_o as head-pair tiles [128, 4, D] so the output projection contracts
        # with K=128 (full PE rows): pair p rows = heads 2p (0:64), 2p+1 (64:128)
        WOT = persist.tile([128, 4, D], cdt, tag="wo", name="wo")
        WO = [WOT[:, i, :] for i in range(4)]

        # ones column per head in V (softmax denominator accumulates via PE)
        for lt in range(nlt):
            nc.gpsimd.memset(V[lt][:, :, DK:DK + 1], 1.0)
        nc.sync.dma_start(out=WOT, in_=woT.rearrange("(p r) e -> r p e", r=128))

        # ---- projections ----
        # weights/x loaded with ONE batched 3-D DMA each ([128, ndt, .] view of
        # the [D, .] dram tensor): the SP sequencer pays ~1.2us per dma_start,
        # so 8 separate [128,512] loads cost ~10us of serialized SEQ time vs
        # ~1.5us for the batched form.
        def load_w(which, wdram, rep=0):
            w = wpool.tile([128, ndt, DL], cdt, tag=f"w{which}", name=f"w_{which}_{rep}")
            nc.sync.dma_start(
                out=w, in_=wdram.rearrange("(t r) e -> r t e", r=128))
            return w

        def load_x(which, xdram, lb, rep=0):
            x = xpool.tile([128, ndt, 512], cdt, tag="x", name=f"x_{which}{lb}_{rep}")
            nc.sync.dma_start(
                out=x, in_=xdram[:, ts(lb, 512)].rearrange("(t r) j -> r t j", r=128))
            return x

        def project_half(which, xdram, lb, half, rep, wt, xcache):
            """one accumulator's worth (~2.1us of PE) of a projection block.
            Fillers must stay under ACT's ~2.5us PSUM backlog or they starve
            the exp stream, so projections are injected at this granularity.
            The x tile is loaded by whichever half runs first."""
            key = (which, lb, rep)
            if key not in xcache:
                xcache[key] = load_x(which, xdram, lb, rep)
            xt = xcache[key]
            if which in ("q", "k"):
                dst = QT if which == "q" else KT
                eth = half
                ps = spsum(f"ps_{which}{lb}_{eth}_{rep}")
                for t in range(ndt):
                    for eh in range(2):
                        et = eth * 2 + eh
                        nc.tensor.matmul(ps[:, ts(eh, 512)],
                                         lhsT=wt[:, t, ts(et, 128)],
                                         rhs=xt[:, t, :],
                                         start=(t == 0), stop=(t == ndt - 1))
                for eh in range(2):
                    nc.vector.tensor_copy(dst[eth * 2 + eh][:, ts(lb, 512)],
                                          ps[:, ts(eh, 512)])
            else:
                jh = half
                ps = spsum(f"ps_v{lb}_{jh}_{rep}")
                for t in range(ndt):
                    for jj in range(2):
                        j = jh * 2 + jj
                        nc.tensor.matmul(ps[:, ts(jj, 512)],
                                         lhsT=xt[:, t, ts(j, 128)],
                                         rhs=wt[:, t, :],
                                         start=(t == 0), stop=(t == ndt - 1))
                for jj in range(2):
                    lt = lb * 4 + jh * 2 + jj
                    nc.vector.tensor_copy(
                        V[lt][:, :, 0:DK],
                        ps[:, ts(jj, 512)].rearrange("p (h d) -> p h d", h=HLOC))

        def project(which, xdram, wdram, lbs, rep=0, wt=None, xcache=None):
            if wt is None:
                wt = load_w(which, wdram, rep)
            if xcache is None:
                xcache = {}
            for lb in lbs:
                for half in range(2):
                    project_half(which, xdram, lb, half, rep, wt, xcache)

        # ---- attention: software-pipelined scores/exp/PV per head-pair ----
        # Score chunks ([128,512] matmul outputs) accumulate into rotating
        # [128,1536] PSUM supertiles; one wide ACT exp fires per filled tile
        # (amortizing ACT's ~280-cycle per-instruction overhead over 3 chunks
        # instead of 2). PV reads 512-wide slices of the resulting P tiles.
        ngrp = nlt // 2

        class SStream:
            """per-pair score-chunk stream. flush() must be called before any
            other use of the "st" PSUM tag (proj/outproj), else the in-order
            PE queue deadlocks on the open supertile's pending exp."""

            def __init__(self, name):
                self.name = name
                self.st = None
                self.filled = 0
                self.nflush = 0
                self.refs = {}      # chunk id -> (ptile, slot)
                self.c = 0

            def add(self, cid, mm_fn):
                if self.st is None:
                    self.st = spsum(f"st_{self.name}_{self.c}")
                    self.filled = 0
                mm_fn(self.st[:, ts(self.filled, 512)])
                self.refs[cid] = (self.st, self.filled)  # placeholder until flush
                self.filled += 1
                self.c += 1
                if self.filled == s_cap:
                    self.flush()

            def flush(self):
                if self.st is None:
                    return
                w = self.filled * 512
                nb = {1: 4, 2: 6 if wide else 26, 3: 12}[self.filled]
                pt = ppool.tile([128, w], cdt, tag=f"pt{self.filled}", bufs=nb,
                                name=f"pt_{self.name}_{self.nflush}")
                nc.scalar.activation(pt, self.st[:, 0:w],
                                     mybir.ActivationFunctionType.Exp, scale=SCALE)
                for cid, (st, slot) in list(self.refs.items()):
                    if st is self.st:
                        self.refs[cid] = (pt, slot)
                self.st = None
                self.nflush += 1

        def s_group(mb, p, g, stream):
            """emit score chunk matmuls for one key-group into the stream;
            chunk order A0,B0,A1,B1 alternates PE row-halves so consecutive
            chunks overlap in the array. Returns the group's chunk ids."""
            cids = []
            for j in (0, 1):
                nt = 2 * g + j
                for h, rows, pos in ((0, slice(0, 64), (0, 0)),
                                     (1, slice(64, 128), (64, 0))):
                    cid = (g, h, j)
                    kt, qt = KT[p][rows, ts(nt, 128)], QT[p][rows, ts(mb, 512)]

                    def mm(out, kt=kt, qt=qt, pos=pos):
                        nc.tensor.matmul(out, lhsT=kt, rhs=qt,
                                         start=True, stop=True, tile_position=pos)
                    stream.add(cid, mm)
                    cids.append(cid)
            return g, stream, cids

        def pv_group(oA, oB, p, item):
            g, stream, _ = item
            hA, hB = 2 * p, 2 * p + 1
            for j in (0, 1):
                nt = 2 * g + j
                pa, sa = stream.refs[(g, 0, j)]
                pb, sb = stream.refs[(g, 1, j)]
                nc.tensor.matmul(oA, lhsT=V[nt][:, hA, :], rhs=pa[:, ts(sa, 512)],
                                 start=(nt == 0), stop=(nt == nlt - 1))
                nc.tensor.matmul(oB, lhsT=V[nt][:, hB, :], rhs=pb[:, ts(sb, 512)],
                                 start=(nt == 0), stop=(nt == nlt - 1))

        def normalize(mb, p, oA, oB, ots):
            """divide by the ones-row sums; assemble [128,512] O^T pair tile."""
            hA, hB = 2 * p, 2 * p + 1
            otp = otpool.tile([128, 512], cdt, tag="ot", name=f"otp_{mb}_{p}")
            for o, h in ((oA, hA), (oB, hB)):
                r = small.tile([1, 512], f32, tag="r", name=f"r_{mb}_{h}")
                # NOTE: reciprocal_approx_fast returned garbage here (rel err
                # 1e5; PSUM src and/or [1,512] single-partition AP hits its
                # undefined path) -- keep the exact iterative reciprocal.
                nc.vector.reciprocal(r, o[DK:DK + 1, :])
                rb = small.tile([64, 512], f32, tag="rb", name=f"rb_{mb}_{h}")
                nc.gpsimd.partition_broadcast(rb, r)
                if h == hA:
                    nc.vector.tensor_mul(otp[0:DK, :], o[0:DK, :], rb)
                else:
                    tmp = small.tile([64, 512], cdt, tag="obt", name=f"obt_{mb}_{h}")
                    nc.vector.tensor_mul(tmp, o[0:DK, :], rb)
                    # partition move 0:64 -> 64:128 needs a DMA; issue it from
                    # the idle Pool sequencer (SP pays ~565ns per dma_start)
                    nc.gpsimd.dma_start(out=otp[64:128, :], in_=tmp)
            ots[p] = otp

        # ---- global software pipeline across ALL (mb, p) pairs ----
        # pending pv-groups survive pair/mb boundaries (per-block pipeline
        # drains idled ACT ~5-9us each, ~14x per rep). Fillers (V/Q
        # projections, out-projections) are injected between score groups;
        # with s_cap=2 every group boundary has the "st" PSUM tag flushed,
        # so filler spsum allocation is always legal there.
        pending = []

        def drain_one():
            poA, poB, pmb, pp, pots, pitem = pending.pop(0)
            pv_group(poA, poB, pp, pitem)
            if pitem[0] == ngrp - 1:
                normalize(pmb, pp, poA, poB, pots)

        def attn_block(mb, plist, ots, head=None, fillers=()):
            """emit score groups for pairs in plist, draining the global
            pending window (lag 3). head: (stream, items) of PRE-EMITTED
            s-groups for plist[0] -- only their pv-groups are seeded into
            pending here. fillers: callables injected one-per-group from
            g=1 (and flushed at the end of the block)."""
            fillers = list(fillers)
            for p in plist:
                if head is not None and p == plist[0]:
                    if VARIANT.startswith("nopv"):
                        continue
                    oA = ps_pv.tile([DK + 1, 512], f32, tag="pv", name=f"oA_{mb}_{p}")
                    oB = ps_pv.tile([DK + 1, 512], f32, tag="pv", name=f"oB_{mb}_{p}")
                    for item in head[1]:
                        pending.append((oA, oB, mb, p, ots, item))
                    continue
                stream = SStream(f"s{mb}_{p}")
                if VARIANT.startswith("nopv"):
                    for g in range(ngrp):
                        s_group(mb, p, g, stream)
                    stream.flush()
                    continue
                oA = ps_pv.tile([DK + 1, 512], f32, tag="pv", name=f"oA_{mb}_{p}")
                oB = ps_pv.tile([DK + 1, 512], f32, tag="pv", name=f"oB_{mb}_{p}")
                for g in range(ngrp):
                    item = s_group(mb, p, g, stream)
                    if g >= 1 and fillers:
                        stream.flush()   # "st" tag must be free for filler PSUM
                        fillers.pop(0)()
                    if len(pending) >= 3:
                        drain_one()
                    pending.append((oA, oB, mb, p, ots, item))
                stream.flush()
            while fillers:
                fillers.pop(0)()

        def outproj_unit(mb, ots, eth):
            po = spsum(f"po_{mb}_{eth}")
            for eh in range(2):
                et = eth * 2 + eh
                for p in range(4):
                    nc.tensor.matmul(po[:, ts(eh, 512)],
                                     lhsT=WO[p][:, ts(et, 128)], rhs=ots[p],
                                     start=(p == 0), stop=(p == 3))
            ost = opool.tile([128, 1024], cdt, tag="ostage", name=f"ost_{mb}_{eth}")
            nc.vector.tensor_copy(ost, po[:, 0:1024])
            # both eh slices in one batched 3-D DMA, issued from Pool
            nc.gpsimd.dma_start(
                out=outT[ts(eth, 256), ts(mb, 512)].rearrange(
                    "(e r) j -> r e j", r=128),
                in_=ost.rearrange("p (e j) -> p e j", e=2))

        def outproj_units(mb, ots):
            if VARIANT.startswith("nopv"):
                return []
            return [(lambda eth=eth: outproj_unit(mb, ots, eth))
                    for eth in range(ndt // 2)]

        if VARIANT.startswith("null"):
            # timing-baseline body: one tiny output write, no compute
            z = opool.tile([128, 512], cdt, tag="ostage", name="znull")
            nc.vector.memset(z, 0.0)
            nc.sync.dma_start(out=outT[0:128, 0:512], in_=z)
            return
        carry_units = []   # previous rep's final out-projection units
        for rep in range(KREP):
            if VARIANT.startswith("proj"):
                project("k", xkT, wkT, range(nlb), rep)
                project("q", xqT, wqT, [0], rep)
                project("v", xvT, wvT, range(nlb), rep)
                for et in range(4):
                    ost = opool.tile([128, l], cdt, tag="big", name=f"pst_{rep}_{et}")
                    nc.vector.tensor_copy(ost, QT[et])
                    nc.sync.dma_start(out=outT[ts(et, 128), :], in_=ost)
                continue
            # mb0-pair0 score groups are interleaved with the K projection
            # (group g only needs keys from K-block g//2) so ACT starts after
            # ~2 projection blocks instead of after all of K+Q+V; their PV
            # matmuls run after V-proj, holding pair0's P tiles in the ppool.
            # The previous rep's tail (leftover pv-groups of its last pair +
            # its final out-projection) is carried INTO this phase so ACT is
            # never idle across the rep boundary.
            ots = {mb: [None] * 4 for mb in range(nlb)}
            n_carry = len(pending)
            wk = load_w("k", wkT, rep)
            wq = load_w("q", wqT, rep)
            project("k", xkT, wkT, [0], rep, wt=wk)
            project("q", xqT, wqT, [0], rep, wt=wq)
            stream0 = SStream(f"h0_{rep}")
            items0 = []
            drained = 0
            for lb in range(1, nlb):
                items0 += [s_group(0, 0, 2 * (lb - 1), stream0),
                           s_group(0, 0, 2 * lb - 1, stream0)]
                stream0.flush()
                project("k", xkT, wkT, [lb], rep, wt=wk)
                # retire prev-rep pv-groups between K-blocks (keeps ACT's
                # exp backlog from the previous rep flowing into pv work)
                while drained < n_carry and drained < 3 * lb:
                    drain_one()
                    drained += 1
            items0 += [s_group(0, 0, 2 * (nlb - 1), stream0),
                       s_group(0, 0, 2 * nlb - 1, stream0)]
            stream0.flush()
            # all prev-rep pv-groups must retire before V tiles are rewritten
            while drained < n_carry:
                drain_one()
                drained += 1
            # pair (0,0) pv-groups seed the global pending window; they drain
            # during pair (0,1)'s score groups. V-block lb must be emitted
            # before the pv-group needing V[2g:2g+2] drains -- lb0 up front,
            # lb1..3 as fillers at groups 1..3 of pair (0,1).
            wv = load_w("v", wvT, rep)
            xc = {}
            project("v", xvT, wvT, [0], rep, wt=wv, xcache=xc)
            attn_block(0, [0], ots[0], head=(stream0, items0))
            # V half-units line up exactly with the drain schedule: pv(0,0,g)
            # drains at pair (0,1)'s group g and needs V[2g:2g+2] = half-unit
            # (lb=g//2, half=g%2), injected one group earlier.
            attn_block(0, [1], ots[0], fillers=[
                (lambda lb=lb, hf=hf: project_half("v", xvT, lb, hf, rep, wv, xc))
                for lb in range(1, nlb) for hf in range(2)])
            attn_block(0, [2], ots[0], fillers=(
                ([(lambda hf=hf: project_half("q", xqT, 1, hf, rep, wq, xc))
                  for hf in range(2)] if nlb > 1 else [])
                + carry_units))
            carry_units = []
            attn_block(0, [3], ots[0])
            for mb in range(1, nlb):
                attn_block(mb, [0], ots[mb])
                # outproj(mb-1) is legal once normalize(mb-1, 3) has fired,
                # which the window-8 lag places at pair (mb, 0)'s tail.
                attn_block(mb, [1], ots[mb], fillers=outproj_units(mb - 1, ots[mb - 1]))
                attn_block(mb, [2], ots[mb], fillers=(
                    [(lambda mbn=mb + 1, hf=hf:
                      project_half("q", xqT, mbn, hf, rep, wq, xc))
                     for hf in range(2)] if mb + 1 < nlb else []))
                attn_block(mb, [3], ots[mb])
            # leave the last pair's pv-groups pending for the next rep's
            # projection phase; stash its out-projection as next-rep filler
            carry_units = outproj_units(nlb - 1, ots[nlb - 1])
        while pending:
            drain_one()
        for u in carry_units:
            u()


_PROGRAMS = {}


def _get_program(krep=None, variant=None):
    key = (krep if krep is not None else KREP,
           variant if variant is not None else VARIANT)
    if key not in _PROGRAMS:
        nc = bacc.Bacc("TRN2", target_bir_lowering=False, debug=False,
                       enable_asserts=False)
        _build(nc, krep=key[0], variant=key[1])
        nc.compile()
        _PROGRAMS[key] = nc
    return _PROGRAMS[key]


def _in_maps(q, k, v, w_q, w_k, w_v, w_o):
    npdt = _NPDT[COMPUTE_DT]
    q, k, v = (np.asarray(a, np.float32) for a in (q, k, v))
    w_q, w_k, w_v, w_o = (np.asarray(a, np.float32) for a in (w_q, w_k, w_v, w_o))
    maps = []
    for core in range(NCORES):
        b, hg = divmod(core, 2)
        hsl = slice(hg * DL, (hg + 1) * DL)
        maps.append({
            "xqT": np.ascontiguousarray(q[b].T).astype(npdt),
            "xkT": np.ascontiguousarray(k[b].T).astype(npdt),
            "xvT": np.ascontiguousarray(v[b].T).astype(npdt),
            "wqT": np.ascontiguousarray(w_q[hsl].T).astype(npdt),
            "wkT": np.ascontiguousarray(w_k[hsl].T).astype(npdt),
            "wvT": np.ascontiguousarray(w_v[hsl].T).astype(npdt),
            "woT": np.ascontiguousarray(w_o[:, hsl].T).astype(npdt),
        })
    return maps


def _run(inputs, **kwargs):
    nc = _get_program()
    maps = _in_maps(**inputs)
    res = run_bass_kernel_spmd(nc, maps, list(range(NCORES)), **kwargs)
    out = np.zeros((B, L, D), np.float32)
    for core in range(NCORES):
        out[core // 2] += np.asarray(res.results[core]["outT"], np.float32).T
    return out, res


def kernel(q, k, v, w_q, w_k, w_v, w_o):
    out, _ = _run(dict(q=q, k=k, v=v, w_q=w_q, w_k=w_k, w_v=w_v, w_o=w_o))
    return out



# revision 20
# speedup vs baseline: 1.8171x; 1.8171x over previous
"""Multi-head attention (B=4, L=2048, D=1024, H=16) on 8 trn2 NeuronCores.

Sharding: core = b*2 + hg  ->  batch sample b in 0..3, head-group hg in 0..1.
Each core handles one batch sample and 8 heads (512 of the 1024 model dims):
data parallel on B, tensor parallel on H (column-split w_q/w_k/w_v, row-split
w_o).  Each core produces a partial output (its 8 heads' contribution through
w_o); the host sums the two head-group partials per sample.

Device-side layouts are all transposed (contraction dim on partitions) so the
kernel needs no on-device transposes:
  xT   [D=1024, L=2048]  (host pre-transposes q/k/v per sample)
  QT/KT [512, 2048]      head-pair tiles: partitions 0:64 head 2p, 64:128 head 2p+1
  V    natural [L, 512]  stored per l-tile as [128, 8 heads, 65] with a ones
                         column appended per head (row sums of P for free)
  S^T  [keys, queries]   per (pair, m-block) in PSUM -> exp on ACT -> P^T bf16
  O^T  = V^T @ P^T accumulated in PSUM; row 64 = sum_n P^T[n, m] (softmax denom)
  outT [1024, 2048]      final partial, host transposes back
Softmax skips max-subtraction: scaled scores are ~N(0,1) (the 1/8 scale is
applied inside the ACT exp); |score| < ~7 over 33M samples, far from fp32 exp
overflow.

The attention loop is ONE GLOBAL software pipeline across all 16 (mb, pair)
units AND across KREP rep boundaries: a pending-window of pv-groups survives
pair/mb boundaries (each pair's PVs drain during the NEXT pair's score
groups, lag ~8 groups), the last pair's pv-groups and the final
out-projection carry into the next rep's K-projection phase, so the PE never
sits in a pipeline flush and ACT (the bottleneck engine: 256 exps of
[128,1024] ~= 322us measured via the nopv variant; the kernel runs at
~256 x 1117ns = the ACT floor at light load) is never starved at
boundaries. V/Q projections and out-projections are injected as filler units
between score groups (legal at every group boundary: with 2-chunk supertiles
the "st" PSUM tag is always flushed there). Score chunks ([128,512] matmul
outputs) accumulate into 3 rotating [128,1024] PSUM tiles (6 banks) shared
with the projection/out-projection accumulators; the PV accumulators take
the remaining 2 banks. The K projection is interleaved with the first pair's
score groups (group g only needs K-block g//2) so ACT starts ~50us earlier.
DMAs are batched 3-D loads ([128, ndt, .] views; one dma_start each for w/x
blocks) and staging/output DMAs issue from the idle Pool sequencer: SP-SEQ
dma_start costs ~565ns+625ns HWDGE each, and the old per-tile loads put
~240us of serialized issue time on SP (172 -> 45 dma instructions).
fp8 (DoubleRow or plain) was evaluated and is numerically DEAD here:
attention outputs are weighted means, so per-element quantization noise
(~6-9% for e4m3) passes ~1:1 to relative output error (measured 4-8e-2
vs the 2e-2 gate). Everything stays bf16 with fp32 PSUM accumulation.
"""

import os
import numpy as np
import ml_dtypes
from contextlib import ExitStack

import concourse.bass as bass
import concourse.tile as tile
from concourse import bacc, mybir
from concourse.bass import ts
from concourse.bass_utils import run_bass_kernel_spmd

B, L, D, H = 4, 2048, 1024, 16
DK = 64          # head dim
HLOC = 8         # heads per core
DL = 512         # model dims per core (HLOC * DK)
SCALE = 1.0 / 8.0
NCORES = 8

COMPUTE_DT = os.environ.get("KDT", "bf16")   # 'bf16' | 'f32'
VARIANT = os.environ.get("KVARIANT", "full")  # 'full' | 'proj' | 'nopv'
KREP = int(os.environ.get("KREP", "1"))       # body repetitions (timing only)

_MDT = {"bf16": mybir.dt.bfloat16, "f32": mybir.dt.float32}
_NPDT = {"bf16": ml_dtypes.bfloat16, "f32": np.float32}


def _build(nc, l=L, krep=None, variant=None):
    global KREP, VARIANT
    if krep is not None:
        KREP = krep
    if variant is not None:
        VARIANT = variant
    cdt = _MDT[COMPUTE_DT]
    f32 = mybir.dt.float32
    nlt = l // 128       # l-tiles of 128
    nlb = l // 512       # l-blocks of 512
    ndt = D // 128       # contraction d-tiles for projections

    xqT = nc.dram_tensor("xqT", [D, l], cdt, kind="ExternalInput").ap()
    xkT = nc.dram_tensor("xkT", [D, l], cdt, kind="ExternalInput").ap()
    xvT = nc.dram_tensor("xvT", [D, l], cdt, kind="ExternalInput").ap()
    wqT = nc.dram_tensor("wqT", [D, DL], cdt, kind="ExternalInput").ap()
    wkT = nc.dram_tensor("wkT", [D, DL], cdt, kind="ExternalInput").ap()
    wvT = nc.dram_tensor("wvT", [D, DL], cdt, kind="ExternalInput").ap()
    woT = nc.dram_tensor("woT", [DL, D], cdt, kind="ExternalInput").ap()
    outT = nc.dram_tensor("outT", [D, l], cdt, kind="ExternalOutput").ap()

    with tile.TileContext(nc) as tc, ExitStack() as ctx:
        persist = ctx.enter_context(tc.tile_pool(name="persist", bufs=1))
        wpool = ctx.enter_context(tc.tile_pool(name="wpool", bufs=2))
        xpool = ctx.enter_context(tc.tile_pool(name="xpool", bufs=2))
        ppool = ctx.enter_context(tc.tile_pool(name="ppool", bufs=4))
        otpool = ctx.enter_context(tc.tile_pool(name="otpool", bufs=7))
        opool = ctx.enter_context(tc.tile_pool(name="opool", bufs=3))
        small = ctx.enter_context(tc.tile_pool(name="small", bufs=4))
        # 2 rotating [128,1536] f32 supertiles (6 banks): score chunks,
        # projection and out-projection accumulators all share this tag.
        ps_s = ctx.enter_context(tc.tile_pool(name="ps_s", bufs=2, space="PSUM"))
        ps_pv = ctx.enter_context(tc.tile_pool(name="ps_pv", bufs=2, space="PSUM"))

        # default: 3 x [128,1024] score tiles, exp per 2 chunks. The wider
        # 2 x [128,1536] / exp-per-3 layout ("w" variant suffix) cuts ACT
        # instruction overhead ~25% but measured slower end-to-end (the
        # 2-buffer PSUM rotation couples PE to exp latency too tightly).
        wide = VARIANT.endswith("w")
        s_shape = [128, 1536] if wide else [128, 1024]
        s_bufs = 2 if wide else 3
        s_cap = 3 if wide else 2

        def spsum(name):
            return ps_s.tile(s_shape, f32, tag="st", bufs=s_bufs, name=name)

        # persistent tiles
        QT = [persist.tile([128, l], cdt, tag=f"qt{i}", name=f"qt{i}") for i in range(4)]
        KT = [persist.tile([128, l], cdt, tag=f"kt{i}", name=f"kt{i}") for i in range(4)]
        V = [persist.tile([128, HLOC, DK + 1], cdt, tag=f"v{i}", name=f"v{i}")
             for i in range(nlt)]
        # w_o as head-pair tiles [128, 4, D] so the output projection contracts
        # with K=128 (full PE rows): pair p rows = heads 2p (0:64), 2p+1 (64:128)
        WOT = persist.tile([128, 4, D], cdt, tag="wo", name="wo")
        WO = [WOT[:, i, :] for i in range(4)]

        # ones column per head in V (softmax denominator accumulates via PE)
        for lt in range(nlt):
            nc.gpsimd.memset(V[lt][:, :, DK:DK + 1], 1.0)
        nc.sync.dma_start(out=WOT, in_=woT.rearrange("(p r) e -> r p e", r=128))

        # ---- projections ----
        # weights/x loaded with ONE batched 3-D DMA each ([128, ndt, .] view of
        # the [D, .] dram tensor): the SP sequencer pays ~1.2us per dma_start,
        # so 8 separate [128,512] loads cost ~10us of serialized SEQ time vs
        # ~1.5us for the batched form.
        def load_w(which, wdram, rep=0):
            w = wpool.tile([128, ndt, DL], cdt, tag=f"w{which}", name=f"w_{which}_{rep}")
            nc.sync.dma_start(
                out=w, in_=wdram.rearrange("(t r) e -> r t e", r=128))
            return w

        def load_x(which, xdram, lb, rep=0):
            x = xpool.tile([128, ndt, 512], cdt, tag="x", name=f"x_{which}{lb}_{rep}")
            nc.sync.dma_start(
                out=x, in_=xdram[:, ts(lb, 512)].rearrange("(t r) j -> r t j", r=128))
            return x

        def project_half(which, xdram, lb, half, rep, wt, xcache):
            """one accumulator's worth (~2.1us of PE) of a projection block.
            Fillers must stay under ACT's ~2.5us PSUM backlog or they starve
            the exp stream, so projections are injected at this granularity.
            The x tile is loaded by whichever half runs first."""
            key = (which, lb, rep)
            if key not in xcache:
                xcache[key] = load_x(which, xdram, lb, rep)
            xt = xcache[key]
            if which in ("q", "k"):
                dst = QT if which == "q" else KT
                eth = half
                ps = spsum(f"ps_{which}{lb}_{eth}_{rep}")
                for t in range(ndt):
                    for eh in range(2):
                        et = eth * 2 + eh
                        nc.tensor.matmul(ps[:, ts(eh, 512)],
                                         lhsT=wt[:, t, ts(et, 128)],
                                         rhs=xt[:, t, :],
                                         start=(t == 0), stop=(t == ndt - 1))
                for eh in range(2):
                    nc.vector.tensor_copy(dst[eth * 2 + eh][:, ts(lb, 512)],
                                          ps[:, ts(eh, 512)])
            else:
                jh = half
                ps = spsum(f"ps_v{lb}_{jh}_{rep}")
                for t in range(ndt):
                    for jj in range(2):
                        j = jh * 2 + jj
                        nc.tensor.matmul(ps[:, ts(jj, 512)],
                                         lhsT=xt[:, t, ts(j, 128)],
                                         rhs=wt[:, t, :],
                                         start=(t == 0), stop=(t == ndt - 1))
                for jj in range(2):
                    lt = lb * 4 + jh * 2 + jj
                    nc.vector.tensor_copy(
                        V[lt][:, :, 0:DK],
                        ps[:, ts(jj, 512)].rearrange("p (h d) -> p h d", h=HLOC))

        def project(which, xdram, wdram, lbs, rep=0, wt=None, xcache=None):
            if wt is None:
                wt = load_w(which, wdram, rep)
            if xcache is None:
                xcache = {}
            for lb in lbs:
                for half in range(2):
                    project_half(which, xdram, lb, half, rep, wt, xcache)

        # ---- attention: software-pipelined scores/exp/PV per head-pair ----
        # Score chunks ([128,512] matmul outputs) accumulate into rotating
        # [128,1536] PSUM supertiles; one wide ACT exp fires per filled tile
        # (amortizing ACT's ~280-cycle per-instruction overhead over 3 chunks
        # instead of 2). PV reads 512-wide slices of the resulting P tiles.
        ngrp = nlt // 2

        class SStream:
            """per-pair score-chunk stream. flush() must be called before any
            other use of the "st" PSUM tag (proj/outproj), else the in-order
            PE queue deadlocks on the open supertile's pending exp."""

            def __init__(self, name):
                self.name = name
                self.st = None
                self.filled = 0
                self.nflush = 0
                self.refs = {}      # chunk id -> (ptile, slot)
                self.c = 0

            def add(self, cid, mm_fn):
                if self.st is None:
                    self.st = spsum(f"st_{self.name}_{self.c}")
                    self.filled = 0
                mm_fn(self.st[:, ts(self.filled, 512)])
                self.refs[cid] = (self.st, self.filled)  # placeholder until flush
                self.filled += 1
                self.c += 1
                if self.filled == s_cap:
                    self.flush()

            def flush(self):
                if self.st is None:
                    return
                w = self.filled * 512
                nb = {1: 4, 2: 6 if wide else 26, 3: 12}[self.filled]
                pt = ppool.tile([128, w], cdt, tag=f"pt{self.filled}", bufs=nb,
                                name=f"pt_{self.name}_{self.nflush}")
                nc.scalar.activation(pt, self.st[:, 0:w],
                                     mybir.ActivationFunctionType.Exp, scale=SCALE)
                for cid, (st, slot) in list(self.refs.items()):
                    if st is self.st:
                        self.refs[cid] = (pt, slot)
                self.st = None
                self.nflush += 1

        def s_group(mb, p, g, stream):
            """emit score chunk matmuls for one key-group into the stream;
            chunk order A0,B0,A1,B1 alternates PE row-halves so consecutive
            chunks overlap in the array. Returns the group's chunk ids."""
            cids = []
            for j in (0, 1):
                nt = 2 * g + j
                for h, rows, pos in ((0, slice(0, 64), (0, 0)),
                                     (1, slice(64, 128), (64, 0))):
                    cid = (g, h, j)
                    kt, qt = KT[p][rows, ts(nt, 128)], QT[p][rows, ts(mb, 512)]

                    def mm(out, kt=kt, qt=qt, pos=pos):
                        nc.tensor.matmul(out, lhsT=kt, rhs=qt,
                                         start=True, stop=True, tile_position=pos)
                    stream.add(cid, mm)
                    cids.append(cid)
            return g, stream, cids

        def pv_group(oA, oB, p, item):
            g, stream, _ = item
            hA, hB = 2 * p, 2 * p + 1
            for j in (0, 1):
                nt = 2 * g + j
                pa, sa = stream.refs[(g, 0, j)]
                pb, sb = stream.refs[(g, 1, j)]
                nc.tensor.matmul(oA, lhsT=V[nt][:, hA, :], rhs=pa[:, ts(sa, 512)],
                                 start=(nt == 0), stop=(nt == nlt - 1))
                nc.tensor.matmul(oB, lhsT=V[nt][:, hB, :], rhs=pb[:, ts(sb, 512)],
                                 start=(nt == 0), stop=(nt == nlt - 1))

        def normalize(mb, p, oA, oB, ots):
            """divide by the ones-row sums; assemble [128,512] O^T pair tile."""
            hA, hB = 2 * p, 2 * p + 1
            otp = otpool.tile([128, 512], cdt, tag="ot", name=f"otp_{mb}_{p}")
            for o, h in ((oA, hA), (oB, hB)):
                r = small.tile([1, 512], f32, tag="r", name=f"r_{mb}_{h}")
                # NOTE: reciprocal_approx_fast returned garbage here (rel err
                # 1e5; PSUM src and/or [1,512] single-partition AP hits its
                # undefined path) -- keep the exact iterative reciprocal.
                nc.vector.reciprocal(r, o[DK:DK + 1, :])
                rb = small.tile([64, 512], f32, tag="rb", name=f"rb_{mb}_{h}")
                nc.gpsimd.partition_broadcast(rb, r)
                if h == hA:
                    nc.vector.tensor_mul(otp[0:DK, :], o[0:DK, :], rb)
                else:
                    tmp = small.tile([64, 512], cdt, tag="obt", name=f"obt_{mb}_{h}")
                    nc.vector.tensor_mul(tmp, o[0:DK, :], rb)
                    # partition move 0:64 -> 64:128 needs a DMA; issue it from
                    # the idle Pool sequencer (SP pays ~565ns per dma_start)
                    nc.gpsimd.dma_start(out=otp[64:128, :], in_=tmp)
            ots[p] = otp

        # ---- global software pipeline across ALL (mb, p) pairs ----
        # pending pv-groups survive pair/mb boundaries (per-block pipeline
        # drains idled ACT ~5-9us each, ~14x per rep). Fillers (V/Q
        # projections, out-projections) are injected between score groups;
        # with s_cap=2 every group boundary has the "st" PSUM tag flushed,
        # so filler spsum allocation is always legal there.
        pending = []

        def drain_one():
            poA, poB, pmb, pp, pots, pitem = pending.pop(0)
            pv_group(poA, poB, pp, pitem)
            if pitem[0] == ngrp - 1:
                normalize(pmb, pp, poA, poB, pots)

        def attn_block(mb, plist, ots, head=None, fillers=()):
            """emit score groups for pairs in plist, draining the global
            pending window (lag 3). head: (stream, items) of PRE-EMITTED
            s-groups for plist[0] -- only their pv-groups are seeded into
            pending here. fillers: callables injected one-per-group from
            g=1 (and flushed at the end of the block)."""
            fillers = list(fillers)
            for p in plist:
                if head is not None and p == plist[0]:
                    if VARIANT.startswith("nopv"):
                        continue
                    oA = ps_pv.tile([DK + 1, 512], f32, tag="pv", name=f"oA_{mb}_{p}")
                    oB = ps_pv.tile([DK + 1, 512], f32, tag="pv", name=f"oB_{mb}_{p}")
                    for item in head[1]:
                        pending.append((oA, oB, mb, p, ots, item))
                    continue
                stream = SStream(f"s{mb}_{p}")
                if VARIANT.startswith("nopv"):
                    for g in range(ngrp):
                        s_group(mb, p, g, stream)
                    stream.flush()
                    continue
                oA = ps_pv.tile([DK + 1, 512], f32, tag="pv", name=f"oA_{mb}_{p}")
                oB = ps_pv.tile([DK + 1, 512], f32, tag="pv", name=f"oB_{mb}_{p}")
                for g in range(ngrp):
                    item = s_group(mb, p, g, stream)
                    if g >= 1 and fillers:
                        stream.flush()   # "st" tag must be free for filler PSUM
                        fillers.pop(0)()
                    if len(pending) >= 3:
                        drain_one()
                    pending.append((oA, oB, mb, p, ots, item))
                stream.flush()
            while fillers:
                fillers.pop(0)()

        def outproj_unit(mb, ots, eth):
            po = spsum(f"po_{mb}_{eth}")
            for eh in range(2):
                et = eth * 2 + eh
                for p in range(4):
                    nc.tensor.matmul(po[:, ts(eh, 512)],
                                     lhsT=WO[p][:, ts(et, 128)], rhs=ots[p],
                                     start=(p == 0), stop=(p == 3))
            ost = opool.tile([128, 1024], cdt, tag="ostage", name=f"ost_{mb}_{eth}")
            nc.vector.tensor_copy(ost, po[:, 0:1024])
            # both eh slices in one batched 3-D DMA, issued from Pool
            nc.gpsimd.dma_start(
                out=outT[ts(eth, 256), ts(mb, 512)].rearrange(
                    "(e r) j -> r e j", r=128),
                in_=ost.rearrange("p (e j) -> p e j", e=2))

        def outproj_units(mb, ots):
            if VARIANT.startswith("nopv"):
                return []
            return [(lambda eth=eth: outproj_unit(mb, ots, eth))
                    for eth in range(ndt // 2)]

        if VARIANT.startswith("null"):
            # timing-baseline body: one tiny output write, no compute
            z = opool.tile([128, 512], cdt, tag="ostage", name="znull")
            nc.vector.memset(z, 0.0)
            nc.sync.dma_start(out=outT[0:128, 0:512], in_=z)
            return
        carry_units = []   # previous rep's final out-projection units
        wk_next = wq_next = xc_next = None  # hoisted next-rep lb0 projections
        for rep in range(KREP):
            if VARIANT.startswith("proj"):
                project("k", xkT, wkT, range(nlb), rep)
                project("q", xqT, wqT, [0], rep)
                project("v", xvT, wvT, range(nlb), rep)
                for et in range(4):
                    ost = opool.tile([128, l], cdt, tag="big", name=f"pst_{rep}_{et}")
                    nc.vector.tensor_copy(ost, QT[et])
                    nc.sync.dma_start(out=outT[ts(et, 128), :], in_=ost)
                continue
            # mb0-pair0 score groups are interleaved with the K projection
            # (group g only needs keys from K-block g//2) so ACT starts after
            # ~2 projection blocks instead of after all of K+Q+V; their PV
            # matmuls run after V-proj, holding pair0's P tiles in the ppool.
            # The previous rep's tail (leftover pv-groups of its last pair +
            # its final out-projection) is carried INTO this phase so ACT is
            # never idle across the rep boundary.
            ots = {mb: [None] * 4 for mb in range(nlb)}
            n_carry = len(pending)
            if wk_next is None:
                # first rep: full K/Q lb0 projections up front
                wk = load_w("k", wkT, rep)
                wq = load_w("q", wqT, rep)
                xc0 = {}
                project("k", xkT, wkT, [0], rep, wt=wk, xcache=xc0)
                project("q", xqT, wqT, [0], rep, wt=wq, xcache=xc0)
            else:
                # q lb0 + k lb0 eth0 were hoisted into the previous rep's
                # tail (their QT/KT mb0-column readers all finish early);
                # only k lb0 eth1 (KT[2],KT[3], read by this rep's pairs
                # p>=2) remains at the boundary.
                wk, wq, xc0 = wk_next, wq_next, xc_next
                project_half("k", xkT, 0, 1, rep, wk, xc0)
            stream0 = SStream(f"h0_{rep}")
            items0 = []
            drained = 0
            for lb in range(1, nlb):
                items0 += [s_group(0, 0, 2 * (lb - 1), stream0),
                           s_group(0, 0, 2 * lb - 1, stream0)]
                stream0.flush()
                project("k", xkT, wkT, [lb], rep, wt=wk)
                # retire prev-rep pv-groups between K-blocks (keeps ACT's
                # exp backlog from the previous rep flowing into pv work)
                while drained < n_carry and drained < 3 * lb:
                    drain_one()
                    drained += 1
            items0 += [s_group(0, 0, 2 * (nlb - 1), stream0),
                       s_group(0, 0, 2 * nlb - 1, stream0)]
            stream0.flush()
            # all prev-rep pv-groups must retire before V tiles are rewritten
            while drained < n_carry:
                drain_one()
                drained += 1
            # pair (0,0) pv-groups seed the global pending window; they drain
            # during pair (0,1)'s score groups. V-block lb must be emitted
            # before the pv-group needing V[2g:2g+2] drains -- lb0 up front,
            # lb1..3 as fillers at groups 1..3 of pair (0,1).
            wv = load_w("v", wvT, rep)
            xc = {}
            project("v", xvT, wvT, [0], rep, wt=wv, xcache=xc)
            attn_block(0, [0], ots[0], head=(stream0, items0))
            # V half-units line up exactly with the drain schedule: pv(0,0,g)
            # drains at pair (0,1)'s group g and needs V[2g:2g+2] = half-unit
            # (lb=g//2, half=g%2), injected one group earlier.
            attn_block(0, [1], ots[0], fillers=[
                (lambda lb=lb, hf=hf: project_half("v", xvT, lb, hf, rep, wv, xc))
                for lb in range(1, nlb) for hf in range(2)])
            attn_block(0, [2], ots[0], fillers=(
                ([(lambda hf=hf: project_half("q", xqT, 1, hf, rep, wq, xc))
                  for hf in range(2)] if nlb > 1 else [])
                + carry_units))
            carry_units = []
            attn_block(0, [3], ots[0])
            for mb in range(1, nlb):
                attn_block(mb, [0], ots[mb])
                # outproj(mb-1) is legal once normalize(mb-1, 3) has fired,
                # which the window-8 lag places at pair (mb, 0)'s tail.
                attn_block(mb, [1], ots[mb], fillers=outproj_units(mb - 1, ots[mb - 1]))
                if mb + 1 < nlb:
                    f2, f3 = [(lambda mbn=mb + 1, hf=hf:
                               project_half("q", xqT, mbn, hf, rep, wq, xc))
                              for hf in range(2)], []
                elif rep + 1 < KREP and not VARIANT.startswith("nopv"):
                    # hoist next rep's q-lb0 halves + k-lb0 eth0 into this
                    # rep's tail (no remaining readers of those QT/KT cols)
                    wk_next = load_w("k", wkT, rep + 1)
                    wq_next = load_w("q", wqT, rep + 1)
                    xc_next = {}
                    f2 = [(lambda hf=hf: project_half(
                        "q", xqT, 0, hf, rep + 1, wq_next, xc_next))
                        for hf in range(2)]
                    f3 = [lambda: project_half("k", xkT, 0, 0, rep + 1,
                                               wk_next, xc_next)]
                else:
                    wk_next = wq_next = xc_next = None
                    f2, f3 = [], []
                attn_block(mb, [2], ots[mb], fillers=f2)
                attn_block(mb, [3], ots[mb], fillers=f3)
            # leave the last pair's pv-groups pending for the next rep's
            # projection phase; stash its out-projection as next-rep filler
            carry_units = outproj_units(nlb - 1, ots[nlb - 1])
        while pending:
            drain_one()
        for u in carry_units:
            u()


_PROGRAMS = {}


def _get_program(krep=None, variant=None):
    key = (krep if krep is not None else KREP,
           variant if variant is not None else VARIANT)
    if key not in _PROGRAMS:
        nc = bacc.Bacc("TRN2", target_bir_lowering=False, debug=False,
                       enable_asserts=False)
        _build(nc, krep=key[0], variant=key[1])
        nc.compile()
        _PROGRAMS[key] = nc
    return _PROGRAMS[key]


def _in_maps(q, k, v, w_q, w_k, w_v, w_o):
    npdt = _NPDT[COMPUTE_DT]
    q, k, v = (np.asarray(a, np.float32) for a in (q, k, v))
    w_q, w_k, w_v, w_o = (np.asarray(a, np.float32) for a in (w_q, w_k, w_v, w_o))
    maps = []
    for core in range(NCORES):
        b, hg = divmod(core, 2)
        hsl = slice(hg * DL, (hg + 1) * DL)
        maps.append({
            "xqT": np.ascontiguousarray(q[b].T).astype(npdt),
            "xkT": np.ascontiguousarray(k[b].T).astype(npdt),
            "xvT": np.ascontiguousarray(v[b].T).astype(npdt),
            "wqT": np.ascontiguousarray(w_q[hsl].T).astype(npdt),
            "wkT": np.ascontiguousarray(w_k[hsl].T).astype(npdt),
            "wvT": np.ascontiguousarray(w_v[hsl].T).astype(npdt),
            "woT": np.ascontiguousarray(w_o[:, hsl].T).astype(npdt),
        })
    return maps


def _run(inputs, **kwargs):
    nc = _get_program()
    maps = _in_maps(**inputs)
    res = run_bass_kernel_spmd(nc, maps, list(range(NCORES)), **kwargs)
    out = np.zeros((B, L, D), np.float32)
    for core in range(NCORES):
        out[core // 2] += np.asarray(res.results[core]["outT"], np.float32).T
    return out, res


def kernel(q, k, v, w_q, w_k, w_v, w_o):
    out, _ = _run(dict(q=q, k=k, v=v, w_q=w_q, w_k=w_k, w_v=w_v, w_o=w_o))
    return out

